# revision 20
# baseline (speedup 1.0000x reference)
"""Deformable sampling module (DCN-style bilinear gather + mask-weighted
tap accumulation) for Trainium2, 8 NeuronCores, data-parallel over batch.

Shapes (hardcoded): input [8, 256, 64, 64], offset [8, 72, 64, 64],
mask [8, 36, 64, 64] -> output [8, 256, 64, 64].
G=4 deformable groups, K=9 taps, Cg=64 channels/group.

v5: ap_gather with a 4-way channel fold.  The Q7 gather ucode pays a
fixed ~102-cycle SBUF read-command latency per index pair, so cost is
dominated by index count, not bytes.  v3 walked every (tap, position)
index once per 16-channel Q7 core (4 cores per group -> 4x redundant).
v5 packs 4 channels into each partition's data array: partition
P = 32g + 16h + p16 holds channels {p16, p16+16, p16+32, p16+48} of
group g as [4096 q, 4 c4, 4 corner] bf16 (d=16 per index), and the two
cores per group split positions in half.  Each (tap, position) index is
walked exactly once -> ~4x fewer read commands.

Weights (tent products x mask) are computed per (k, chunk) in a
p16-replicated layout (inputs host-replicated), applied on DVE in 4x
mode with a stride-0 broadcast over c4; corner pairs fold in-place;
the horizontal pair + 9-tap sum run on the idle PE as identity matmuls
accumulating in PSUM (f32).  Patch base = round(p - 0.5) clamped to
[0, 62]; tents at the patch's integer positions are exact under
clamping (out-of-patch points get zero tent weight).
"""
import contextlib
import sys
import numpy as np
import ml_dtypes

sys.path.insert(0, "/opt/trn_rl_repo")

import concourse.bacc as bacc
import concourse.tile as tile
import concourse.mybir as mybir
from concourse import library_config
from concourse.vector_clock import ScopedClock
from concourse.bass_utils import run_bass_kernel_spmd

F32 = mybir.dt.float32
F16 = mybir.dt.float16
BF16 = mybir.dt.bfloat16
I16 = mybir.dt.int16
OP = mybir.AluOpType
AF = mybir.ActivationFunctionType

B, C, H, W = 8, 256, 64, 64
G, K, Cg = 4, 9, 64
HW = H * W
NCH = 4                     # position chunks per half
NI = 512                    # indices per (k, chunk) gather
NIW = NI // 16              # 32 wrapped idx cols per (k, chunk)
NIDX = K * NCH * NIW        # 1152 idx cols total
NWC = K * NCH * NI          # 18432 weight cols total
KY = np.arange(3).repeat(3)
KX = np.tile(np.arange(3), 3)
MAGIC = float(3 << 22)      # round-to-nearest magic (ulp=1 zone)


def _patch_tile_drain():
    """walrus rejects >1 sync wait on the tile-exit Drain; spill extras
    onto preceding sync-engine nops."""
    if getattr(tile.TileContext, "_drain_patched", False):
        return

    def _drain_and_barrier(self, tick_clock, wait_clock):
        nc = self.nc
        drain_inst = nc.sync.drain()
        wait_clock.add_sem_waits(
            drain_inst.ins, ScopedClock({None: tick_clock.global_clock})
        )
        si = drain_inst.ins.sync_info
        if si is not None and len(si.on_wait) > 1:
            ow = list(si.on_wait)
            si.on_wait = ow[:1]
            for i in range(1, len(ow)):
                nop = nc.sync.nop(nofuse=True, hint="drain_wait_spill")
                nop.ins.sync_info = mybir.SyncInfo(
                    on_wait=[ow[i]], on_update=[]
                )
        nc.all_engine_barrier()
        assert self.sems is not None
        popped = nc._tile_sem_poison_stack.pop()
        assert popped is self._sem_poison
        nc.clear_and_free_semaphores(list(self.sems.allocated().values()))
        nc.all_engine_barrier()

    tile.TileContext._drain_and_barrier = _drain_and_barrier
    tile.TileContext._drain_patched = True


def _build(loop_n=0):
    _patch_tile_drain()
    nc = bacc.Bacc()

    dD = nc.dram_tensor("dD", [128, HW * 16], BF16, kind="ExternalInput")
    # idx-pipeline inputs, packed [4 quantities][128, 1152].  f16: both
    # pipelines read the same f16 values, so patch bases stay consistent.
    ipack = nc.dram_tensor("ipack", [128, 4 * NIDX], F16,
                           kind="ExternalInput")
    # weight-pipeline inputs, packed per (k, ch): [128, 36, 5, 512]
    wpack = nc.dram_tensor("wpack", [128, K * NCH * 5 * NI], F16,
                           kind="ExternalInput")
    identw = nc.dram_tensor("identw", [128, 128], BF16, kind="ExternalInput")
    # [P, ch, i, c4]; host un-permutes to [C, H, W]
    y = nc.dram_tensor("y", [128, NCH * NI * 4], F32, kind="ExternalOutput")

    with tile.TileContext(nc) as tc:
        nc.gpsimd.load_library(library_config.ap_gather)
        with tc.tile_pool(name="main", bufs=1) as MP:
            loop_cm = tc.For_i(0, loop_n, 1) if loop_n else \
                contextlib.nullcontext()
            with loop_cm:
                ident = MP.tile([128, 128], BF16, tag="ident")
                nc.sync.dma_start(ident[:], identw[:])
                bias0 = MP.tile([128, 1], F32, tag="bias0")
                bias1 = MP.tile([128, 1], F32, tag="bias1")
                biasm1 = MP.tile([128, 1], F32, tag="biasm1")
                nc.vector.memset(bias0[:], 0.0)
                nc.vector.memset(bias1[:], 1.0)
                nc.vector.memset(biasm1[:], -1.0)

                D = MP.tile([128, HW * 16], BF16, tag="D")
                nc.sync.dma_start(D[:], dD[:])
                Dv = D[:].rearrange("p (q d) -> p q d", q=HW, d=16)

                qi16 = MP.tile([128, NIDX], I16, tag="qi16")
                with tc.tile_pool(name="ip", bufs=1) as IP:
                    ipt = IP.tile([128, 4, NIDX], F16, tag="ipt")
                    nc.sync.dma_start(
                        ipt[:], ipack[:].rearrange(
                            "p (a n) -> p a n", a=4, n=NIDX))
                    oy, ox, by, bx = (ipt[:, a, :] for a in range(4))
                    pyp = IP.tile([128, NIDX], F32, tag="pyp")
                    pxp = IP.tile([128, NIDX], F32, tag="pxp")
                    nc.vector.tensor_tensor(pyp[:], oy, by, OP.add)
                    nc.vector.tensor_tensor(pxp[:], ox, bx, OP.add)
                    nc.vector.tensor_scalar(
                        out=pyp[:], in0=pyp[:], scalar1=MAGIC, scalar2=MAGIC,
                        op0=OP.add, op1=OP.subtract)
                    nc.vector.tensor_scalar(
                        out=pxp[:], in0=pxp[:], scalar1=MAGIC, scalar2=MAGIC,
                        op0=OP.add, op1=OP.subtract)
                    nc.vector.tensor_scalar(
                        out=pyp[:], in0=pyp[:], scalar1=0.0, scalar2=62.0,
                        op0=OP.max, op1=OP.min)
                    nc.vector.tensor_scalar(
                        out=pxp[:], in0=pxp[:], scalar1=0.0, scalar2=62.0,
                        op0=OP.max, op1=OP.min)
                    nc.vector.scalar_tensor_tensor(
                        out=pyp[:], in0=pyp[:], scalar=64.0, in1=pxp[:],
                        op0=OP.mult, op1=OP.add)
                    nc.vector.tensor_copy(qi16[:], pyp[:])

                with tc.tile_pool(name="wk", bufs=2) as WK, \
                     tc.tile_pool(name="wt", bufs=1) as WT, \
                     tc.tile_pool(name="w4p", bufs=2) as W4P, \
                     tc.tile_pool(name="outp", bufs=1) as OT, \
                     tc.tile_pool(name="ps", bufs=2, space="PSUM") as PS:
                    for ch in range(NCH):
                        acc = PS.tile([128, NI * 4], F32, tag="acc")
                        for k in range(K):
                            kc = k * NCH + ch
                            # ---- weight slice pipeline ----
                            wsl = WT.tile([128, 5, NI], F16, tag="wsl")
                            nc.sync.dma_start(
                                wsl[:], wpack[:].rearrange(
                                    "p (t a n) -> p t a n",
                                    t=K * NCH, a=5, n=NI)[:, kc])
                            woy, wox, wm, wby, wbx = (
                                wsl[:, a, :] for a in range(5))
                            sh = [128, NI]
                            pyp = WT.tile(sh, F32, tag="wpyp")
                            pxp = WT.tile(sh, F32, tag="wpxp")
                            nc.vector.tensor_tensor(pyp[:], woy, wby, OP.add)
                            nc.vector.tensor_tensor(pxp[:], wox, wbx, OP.add)
                            ybv = WT.tile(sh, F32, tag="wyb")
                            xbv = WT.tile(sh, F32, tag="wxb")
                            nc.vector.tensor_scalar(
                                out=ybv[:], in0=pyp[:], scalar1=MAGIC,
                                scalar2=MAGIC, op0=OP.add, op1=OP.subtract)
                            nc.vector.tensor_scalar(
                                out=xbv[:], in0=pxp[:], scalar1=MAGIC,
                                scalar2=MAGIC, op0=OP.add, op1=OP.subtract)
                            nc.vector.tensor_scalar(
                                out=ybv[:], in0=ybv[:], scalar1=0.0,
                                scalar2=62.0, op0=OP.max, op1=OP.min)
                            nc.vector.tensor_scalar(
                                out=xbv[:], in0=xbv[:], scalar1=0.0,
                                scalar2=62.0, op0=OP.max, op1=OP.min)
                            uy = WT.tile(sh, F32, tag="wuy")
                            ux = WT.tile(sh, F32, tag="wux")
                            nc.vector.scalar_tensor_tensor(
                                out=uy[:], in0=pyp[:], scalar=0.5, in1=ybv[:],
                                op0=OP.add, op1=OP.subtract)
                            nc.vector.scalar_tensor_tensor(
                                out=ux[:], in0=pxp[:], scalar=0.5, in1=xbv[:],
                                op0=OP.add, op1=OP.subtract)
                            ta = WT.tile(sh, F32, tag="wta")
                            ty0 = WT.tile(sh, F32, tag="wty0")
                            ty1 = WT.tile(sh, F32, tag="wty1")
                            # pyp/pxp are dead once uy/ux exist; reuse
                            tx0 = WT.tile(sh, F32, tag="wpyp")
                            tx1 = WT.tile(sh, F32, tag="wpxp")
                            for u, t0, t1 in ((uy, ty0, ty1), (ux, tx0, tx1)):
                                nc.scalar.activation(ta[:], u[:], AF.Abs,
                                                     bias=bias0[:])
                                nc.scalar.activation(
                                    t0[:], ta[:], AF.Relu, scale=-1.0,
                                    bias=bias1[:])
                                nc.scalar.activation(ta[:], u[:], AF.Abs,
                                                     bias=biasm1[:])
                                nc.scalar.activation(
                                    t1[:], ta[:], AF.Relu, scale=-1.0,
                                    bias=bias1[:])
                            nc.vector.tensor_tensor(
                                ty0[:], ty0[:], wm, OP.mult)
                            nc.vector.tensor_tensor(
                                ty1[:], ty1[:], wm, OP.mult)
                            w4 = W4P.tile([128, NI, 4], BF16, tag="w4")
                            nc.vector.tensor_tensor(
                                w4[:, :, 0], ty0[:], tx0[:], OP.mult)
                            nc.vector.tensor_tensor(
                                w4[:, :, 1], ty0[:], tx1[:], OP.mult)
                            nc.vector.tensor_tensor(
                                w4[:, :, 2], ty1[:], tx0[:], OP.mult)
                            nc.vector.tensor_tensor(
                                w4[:, :, 3], ty1[:], tx1[:], OP.mult)

                            # ---- gather ----
                            gt = WK.tile([128, NI, 4, 4], BF16, tag="gt")
                            nc.gpsimd.ap_gather(
                                gt[:].rearrange("p i c e -> p i (c e)"),
                                Dv,
                                qi16[:, kc * NIW:(kc + 1) * NIW],
                                channels=128, num_elems=HW, d=16,
                                num_idxs=NI)

                            # ---- combine ----
                            wbc = w4[:].unsqueeze(2).broadcast_to(
                                [128, NI, 4, 4])
                            nc.vector.tensor_tensor(
                                gt[:], gt[:], wbc, OP.mult)
                            nc.vector.tensor_tensor(
                                gt[:, :, :, 0:2], gt[:, :, :, 0:2],
                                gt[:, :, :, 2:4], OP.add)
                            for e in range(2):
                                for qt in range(4):
                                    nc.tensor.matmul(
                                        acc[:, qt * 512:(qt + 1) * 512],
                                        ident[:],
                                        gt[:, qt * 128:(qt + 1) * 128, :, e],
                                        start=(k == 0 and e == 0),
                                        stop=(k == K - 1 and e == 1))
                        acc_sb = OT.tile([128, NI * 4], F32, tag="acc_sb")
                        nc.scalar.activation(acc_sb[:], acc[:], AF.Copy)
                        nc.sync.dma_start(
                            y[:, ch * NI * 4:(ch + 1) * NI * 4], acc_sb[:])
    nc.finalize()
    return nc


def _host_prep(input_b, offset_b, mask_b, consts):
    x = np.asarray(input_b, dtype=np.float32).reshape(G, Cg, H, W)
    xpad = np.zeros((G, Cg, H + 1, W + 1), dtype=np.float32)
    xpad[:, :, :H, :W] = x
    X4 = np.empty((G, Cg, H, W, 4), dtype=np.float32)
    X4[..., 0] = xpad[:, :, 0:H, 0:W]
    X4[..., 1] = xpad[:, :, 0:H, 1:W + 1]
    X4[..., 2] = xpad[:, :, 1:H + 1, 0:W]
    X4[..., 3] = xpad[:, :, 1:H + 1, 1:W + 1]
    # [g, c4, p16, q, e] -> partition P = 32g+16h+p16 holds [q, c4, e]
    A = X4.reshape(G, 4, 16, HW, 4)          # c = c4*16 + p16
    A2 = A.transpose(0, 2, 3, 1, 4)          # [g, p16, q, c4, e]
    Dh = A2.reshape(G, 1, 16, HW * 16)
    dD = np.ascontiguousarray(
        np.broadcast_to(Dh, (G, 2, 16, HW * 16)).reshape(128, HW * 16)
    ).astype(ml_dtypes.bfloat16)

    off = np.asarray(offset_b, dtype=np.float32).reshape(G, K, 2, HW)
    # idx layout: P = 32g+16h+r, col = k*128 + ch*32 + j,
    #   q = h*2048 + ch*512 + 16j + r
    oi = off.reshape(G, K, 2, 2, NCH, NIW, 16)   # [g,k,d,h,ch,j,r]
    oi = oi.transpose(0, 3, 6, 2, 1, 4, 5)       # [g,h,r,d,k,ch,j]
    oi = oi.reshape(128, 2, NIDX)
    ipack = np.ascontiguousarray(np.concatenate(
        [oi[:, 0], oi[:, 1], consts["byi"], consts["bxi"]],
        axis=1)).astype(np.float16)

    # weight layout: P = 32g+16h+p16 (p16-replicated),
    #   col = ((k*NCH+ch)*5 + a)*NI + i, q = h*2048 + ch*512 + i
    m = np.asarray(mask_b, dtype=np.float32).reshape(G, K, HW)
    ow = off.reshape(G, K, 2, 2, NCH, NI)        # [g,k,d,h,ch,i]
    mw = m.reshape(G, K, 1, 2, NCH, NI)
    src = np.concatenate([ow, mw], axis=2)       # [g,k,3,h,ch,i]
    src = src.transpose(0, 3, 1, 4, 2, 5)        # [g,h,k,ch,3,i]
    w3 = src.reshape(G * 2, 1, K * NCH, 3, NI)
    w5 = np.concatenate(
        [w3, np.broadcast_to(consts["bw"], (G * 2, 1, K * NCH, 2, NI))],
        axis=3)                                  # [gh,1,kc,5,i]
    wpack = np.ascontiguousarray(
        np.broadcast_to(w5, (G * 2, 16, K * NCH, 5, NI))
        .reshape(128, K * NCH * 5 * NI)).astype(np.float16)

    return {
        "dD": dD,
        "ipack": ipack,
        "wpack": wpack,
        "identw": consts["identw"],
    }


def _consts():
    # idx-pipeline base grids (include -PAD and the -0.5 round->floor shift)
    gg = np.arange(G)[:, None, None, None, None, None]
    hh = np.arange(2)[None, :, None, None, None, None]
    rr = np.arange(16)[None, None, :, None, None, None]
    kk = np.arange(K)[None, None, None, :, None, None]
    cc = np.arange(NCH)[None, None, None, None, :, None]
    jj = np.arange(NIW)[None, None, None, None, None, :]
    q = hh * 2048 + cc * 512 + 16 * jj + rr
    byi = (q // 64 + KY[kk] - 1.5 + 0.0 * gg)
    bxi = (q % 64 + KX[kk] - 1.5 + 0.0 * gg)
    byi = np.ascontiguousarray(
        byi.reshape(128, NIDX), dtype=np.float32)
    bxi = np.ascontiguousarray(
        bxi.reshape(128, NIDX), dtype=np.float32)

    # weight-pipeline base grids [1, kc, 2, NI] (bw -> byw, bxw)
    kk = np.arange(K)[:, None, None]
    cc = np.arange(NCH)[None, :, None]
    ii = np.arange(NI)[None, None, :]
    # q = h*2048 + ch*512 + i; h enters per-partition via broadcast below
    bw = np.empty((2, 2, K, NCH, NI), dtype=np.float32)  # [h, d, k, ch, i]
    for h in range(2):
        qv = h * 2048 + cc * 512 + ii
        bw[h, 0] = qv // 64 + KY[kk] - 1.5
        bw[h, 1] = qv % 64 + KX[kk] - 1.5
    bw = bw.transpose(0, 2, 3, 1, 4).reshape(2, K * NCH, 2, NI)
    # expand over g: same for every g -> [G*2, 1, kc, 2, NI]
    bw = np.ascontiguousarray(
        np.broadcast_to(bw[None], (G, 2, K * NCH, 2, NI))
        .reshape(G * 2, 1, K * NCH, 2, NI))

    identw = np.eye(128, dtype=np.float32).astype(ml_dtypes.bfloat16)
    return {"byi": byi, "bxi": bxi, "bw": bw, "identw": identw}


_STATE = {}


def kernel(input, offset, mask):
    if "nc" not in _STATE:
        _STATE["nc"] = _build()
        _STATE["consts"] = _consts()
    nc = _STATE["nc"]
    consts = _STATE["consts"]
    in_maps = [
        _host_prep(np.asarray(input[b]), np.asarray(offset[b]),
                   np.asarray(mask[b]), consts)
        for b in range(B)
    ]
    res = run_bass_kernel_spmd(nc, in_maps, core_ids=list(range(B)))
    # y [P=(g,h,p16), ch, i, c4] -> out [g, c4*16+p16, h*2048+ch*512+i]
    out = np.stack([
        np.asarray(res.results[b]["y"])
        .reshape(G, 2, 16, NCH, NI, 4)
        .transpose(0, 5, 2, 1, 3, 4)
        .reshape(C, H, W)
        for b in range(B)
    ])
    return out
